# revision 3
# baseline (speedup 1.0000x reference)
"""Llama GQA attention (B=2, S=2048, H=4096, 32 q-heads / 8 kv-heads, RoPE,
causal-capable additive mask, returns (attn_output, attn_weights)) on 8
Trainium2 NeuronCores.

Sharding: core c handles batch b = c//4 and kv-head groups 2*(c%4), 2*(c%4)+1
(= 8 contiguous q-heads).  Tensor-parallel across heads per the hint; the
output projection produces per-core partial sums reduced on the host.

Device-side formulation (per core):
  - All matmuls in bf16 with fp32 PSUM accumulation.
  - hidden^T, weights^T pre-transposed/cast on the host so every DMA is
    natural-layout.
  - scores are computed TRANSPOSED: S^T[t, s] = K^T(d,t)^T-contract-Q^T(d,s),
    so the attn@V matmul consumes exp tiles directly (contraction dim t on
    partitions) and no on-chip transpose of the 33M-element attn tensor is
    needed.
  - The additive mask is applied multiplicatively as exp(mask) (host
    precomputed), which is exact for {0, -inf} masks and mathematically
    identical for general finite masks; this also keeps the ACT exp input
    small (no overflow) without a max-subtraction pass.
  - Row sums via a DVE bf16 add-tree over the 16 t-chunks + a ones-matrix
    matmul on PE (which also broadcasts the sum across all 128 partitions);
    1/sum via ACT exp(-ln(sum)).
  - attn_weights are written transposed [h, t, s] in bf16; the host casts to
    f32 and returns a transposed view.
"""

import numpy as np
import ml_dtypes

B, S, H = 2, 2048, 4096
NH, NKV, HD = 32, 8, 128
REP = NH // NKV
SCALE = HD ** -0.5
P = 128
NCORES = 8
KC = H // P            # 32 contraction chunks for projections
TC = S // P            # 16 key/t chunks
SB = 512               # s block for attention phase
NSB = S // SB          # 4
SHALF = S // 2         # 1024, hidden^T streamed in halves
HPC = 8                # q-heads per core
GPC = 2                # kv groups per core

BF16 = ml_dtypes.bfloat16

_PROGRAM = None


def _build_program():
    import concourse.bass as bass
    import concourse.tile as tile
    from concourse import bacc, mybir

    f32 = mybir.dt.float32
    bf16 = mybir.dt.bfloat16
    AF = mybir.ActivationFunctionType

    nc = bacc.Bacc("TRN2", target_bir_lowering=False)

    # ---- I/O ----
    hT = nc.declare_dram_parameter("hT", [KC, P, S], bf16, isOutput=False)
    wqT = nc.declare_dram_parameter("wqT", [KC, P, HPC * HD], bf16, isOutput=False)
    wkT = nc.declare_dram_parameter("wkT", [KC, P, GPC * HD], bf16, isOutput=False)
    wvT = nc.declare_dram_parameter("wvT", [KC, P, GPC * HD], bf16, isOutput=False)
    woT = nc.declare_dram_parameter("woT", [HPC, P, H], bf16, isOutput=False)
    cosT = nc.declare_dram_parameter("cosT", [P, S], bf16, isOutput=False)
    sinT = nc.declare_dram_parameter("sinT", [P, S], bf16, isOutput=False)
    expmT = nc.declare_dram_parameter("expmT", [TC, P, S], bf16, isOutput=False)
    rotT = nc.declare_dram_parameter("rotT", [P, P], bf16, isOutput=False)
    attnT = nc.declare_dram_parameter("attnT", [HPC, TC, P, S], bf16, isOutput=True)
    outp = nc.declare_dram_parameter("outp", [S // P, P, H], f32, isOutput=True)

    with tile.TileContext(nc) as tctx:
        with (
            tctx.tile_pool(name="singles", bufs=1) as singles,
            tctx.tile_pool(name="persist", bufs=1) as persist,
        ):
            rot_sb = singles.tile([P, P], bf16)
            nc.sync.dma_start(out=rot_sb, in_=rotT[:])
            ones_sb = singles.tile([P, P], bf16)
            nc.vector.memset(ones_sb, 1.0)

            # persistent activations (bf16)
            q_sb = persist.tile([P, HPC, S], bf16)    # Q^T rope'd  [d, head, s]
            k_sb = persist.tile([P, GPC, S], bf16)    # K^T rope'd  [d, group, t]
            v_sb = persist.tile([P, TC, GPC * HD], bf16)  # V natural [t_in, chunk, d]

            # ================= Phase 1: projections + RoPE =================
            with (
                tctx.tile_pool(name="ph1_h", bufs=KC) as hpool,
                tctx.tile_pool(name="ph1_wq", bufs=2) as wqpool,
                tctx.tile_pool(name="ph1_wres", bufs=1) as wres,
                tctx.tile_pool(name="ph1_cs", bufs=1) as cspool,
                tctx.tile_pool(name="ph1_tmp", bufs=3) as tmp1,
                tctx.tile_pool(name="ph1_ps", bufs=2, space="PSUM") as pp1,
                tctx.tile_pool(name="ph1_psr", bufs=2, space="PSUM") as ppr,
            ):
                cos_sb = cspool.tile([P, S], bf16)
                nc.sync.dma_start(out=cos_sb, in_=cosT[:])
                sin_sb = cspool.tile([P, S], bf16)
                nc.sync.dma_start(out=sin_sb, in_=sinT[:])
                wk_sb = wres.tile([P, KC, GPC * HD], bf16)
                nc.sync.dma_start(out=wk_sb, in_=wkT[:].rearrange("c p d -> p c d"))
                wv_sb = wres.tile([P, KC, GPC * HD], bf16)
                nc.sync.dma_start(out=wv_sb, in_=wvT[:].rearrange("c p d -> p c d"))

                def rope(dst, psrc, s0):
                    # dst (bf16, [P, SB]) currently holds the raw projection;
                    # psrc is a free psum slot for the rotate matmul.
                    nc.tensor.matmul(psrc, lhsT=rot_sb, rhs=dst, start=True, stop=True)
                    t0 = tmp1.tile([P, SB], bf16, tag="t0")
                    nc.vector.tensor_mul(out=t0, in0=dst, in1=cos_sb[:, s0:s0 + SB])
                    nc.vector.tensor_mul(out=dst, in0=psrc, in1=sin_sb[:, s0:s0 + SB])
                    nc.vector.tensor_add(out=dst, in0=dst, in1=t0)

                for sh in range(2):
                    hcs = []
                    for k in range(KC):
                        hc = hpool.tile([P, SHALF], bf16, tag="hc")
                        nc.sync.dma_start(
                            out=hc, in_=hT[:][k, :, sh * SHALF:(sh + 1) * SHALF])
                        hcs.append(hc)

                    # Q projections (8 head-chunks)
                    for gh in range(HPC):
                        wq_sl = wqpool.tile([P, KC, HD], bf16, tag="wq")
                        nc.sync.dma_start(
                            out=wq_sl,
                            in_=wqT[:][:, :, gh * HD:(gh + 1) * HD].rearrange("c p d -> p c d"))
                        for sb in range(SHALF // SB):
                            s0 = sh * SHALF + sb * SB
                            ps = pp1.tile([P, SB], f32, tag="proj")
                            for k in range(KC):
                                nc.tensor.matmul(
                                    ps, lhsT=wq_sl[:, k, :],
                                    rhs=hcs[k][:, sb * SB:(sb + 1) * SB],
                                    start=(k == 0), stop=(k == KC - 1))
                            dst = q_sb[:, gh, s0:s0 + SB]
                            nc.scalar.activation(out=dst, in_=ps, func=AF.Copy)
                            pr = ppr.tile([P, SB], f32, tag="rot")
                            rope(dst, pr, s0)

                    # K projections (2 groups)
                    for g in range(GPC):
                        for sb in range(SHALF // SB):
                            s0 = sh * SHALF + sb * SB
                            ps = pp1.tile([P, SB], f32, tag="proj")
                            for k in range(KC):
                                nc.tensor.matmul(
                                    ps, lhsT=wk_sb[:, k, g * HD:(g + 1) * HD],
                                    rhs=hcs[k][:, sb * SB:(sb + 1) * SB],
                                    start=(k == 0), stop=(k == KC - 1))
                            dst = k_sb[:, g, s0:s0 + SB]
                            nc.scalar.activation(out=dst, in_=ps, func=AF.Copy)
                            pr = ppr.tile([P, SB], f32, tag="rot")
                            rope(dst, pr, s0)

                    # V projection, natural layout [t, d], both groups at once
                    for tb in range(SHALF // P):
                        tg = sh * (SHALF // P) + tb
                        psv = pp1.tile([P, GPC * HD], f32, tag="projv")
                        for k in range(KC):
                            nc.tensor.matmul(
                                psv, lhsT=hcs[k][:, tb * P:(tb + 1) * P],
                                rhs=wv_sb[:, k, :],
                                start=(k == 0), stop=(k == KC - 1))
                        nc.scalar.activation(out=v_sb[:, tg, :], in_=psv, func=AF.Copy)

            # persistent ctx (allocated after phase-1 pools close)
            with tctx.tile_pool(name="ctxp", bufs=1) as ctxp:
                ctx_sb = ctxp.tile([P, HPC, S], bf16)   # ctx^T [d, head, s]

                # ================= Phase 2: attention =================
                with (
                    tctx.tile_pool(name="ph2_expm", bufs=2) as expmp,
                    tctx.tile_pool(name="ph2_exp", bufs=2) as expp,
                    tctx.tile_pool(name="ph2_tree", bufs=2) as tree,
                    tctx.tile_pool(name="ph2_ps", bufs=3, space="PSUM") as pps,
                    tctx.tile_pool(name="ph2_psc", bufs=2, space="PSUM") as ppc,
                    tctx.tile_pool(name="ph2_pssum", bufs=2, space="PSUM") as ppsum,
                ):
                    for sb4 in range(NSB):
                        s0 = sb4 * SB
                        expm_sb = expmp.tile([P, TC, SB], bf16, tag="expm")
                        nc.sync.dma_start(
                            out=expm_sb,
                            in_=expmT[:][:, :, s0:s0 + SB].rearrange("c p s -> p c s"))
                        for gh in range(HPC):
                            g = gh // REP
                            exp_sb = expp.tile([P, TC, SB], bf16, tag="exp")
                            for t in range(TC):
                                pss = pps.tile([P, SB], f32, tag="scores")
                                nc.tensor.matmul(
                                    pss, lhsT=k_sb[:, g, t * P:(t + 1) * P],
                                    rhs=q_sb[:, gh, s0:s0 + SB],
                                    start=True, stop=True)
                                nc.scalar.activation(
                                    out=exp_sb[:, t, :], in_=pss, func=AF.Exp,
                                    scale=SCALE)
                            # mask (multiplicative exp(mask)) on gpsimd
                            nc.gpsimd.tensor_mul(out=exp_sb, in0=exp_sb, in1=expm_sb)
                            # bf16 add-tree over the 16 t-chunks
                            t1 = tree.tile([P, 8, SB], bf16, tag="t1")
                            nc.vector.tensor_add(t1, exp_sb[:, 0:8, :], exp_sb[:, 8:16, :])
                            t2 = tree.tile([P, 4, SB], bf16, tag="t2")
                            nc.vector.tensor_add(t2, t1[:, 0:4, :], t1[:, 4:8, :])
                            t3 = tree.tile([P, 2, SB], bf16, tag="t3")
                            nc.vector.tensor_add(t3, t2[:, 0:2, :], t2[:, 2:4, :])
                            acc = tree.tile([P, SB], bf16, tag="acc")
                            nc.vector.tensor_add(acc, t3[:, 0, :], t3[:, 1, :])
                            # replicated column sums + 1/sum = exp(-ln(sum))
                            pssum = ppsum.tile([P, SB], f32, tag="sum")
                            nc.tensor.matmul(pssum, lhsT=ones_sb, rhs=acc,
                                             start=True, stop=True)
                            lnt = tree.tile([P, SB], f32, tag="lnt")
                            nc.scalar.activation(out=lnt, in_=pssum, func=AF.Ln)
                            rb = tree.tile([P, SB], bf16, tag="rb")
                            nc.scalar.activation(out=rb, in_=lnt, func=AF.Exp,
                                                 scale=-1.0)
                            # normalize in place (rb broadcast over chunk dim)
                            nc.vector.tensor_mul(
                                out=exp_sb, in0=exp_sb,
                                in1=rb[:, None, :].to_broadcast((P, TC, SB)))
                            # attn @ V  -> ctx^T [d, s]
                            psc = ppc.tile([P, SB], f32, tag="ctx")
                            for t in range(TC):
                                nc.tensor.matmul(
                                    psc, lhsT=v_sb[:, t, g * HD:(g + 1) * HD],
                                    rhs=exp_sb[:, t, :],
                                    start=(t == 0), stop=(t == TC - 1))
                            nc.scalar.activation(out=ctx_sb[:, gh, s0:s0 + SB],
                                                 in_=psc, func=AF.Copy)
                            # write normalized attn (transposed layout)
                            nc.sync.dma_start(
                                out=attnT[:][gh, :, :, s0:s0 + SB].rearrange("c p s -> p c s"),
                                in_=exp_sb)

                # ================= Phase 3: output projection =================
                with (
                    tctx.tile_pool(name="ph3_wo", bufs=2) as wop,
                    tctx.tile_pool(name="ph3_st", bufs=3) as stp,
                    tctx.tile_pool(name="ph3_ps", bufs=2, space="PSUM") as pp3,
                ):
                    for eb in range(H // SB):
                        wo_sl = wop.tile([P, HPC, SB], bf16, tag="wo")
                        nc.sync.dma_start(
                            out=wo_sl,
                            in_=woT[:][:, :, eb * SB:(eb + 1) * SB].rearrange("c p e -> p c e"))
                        for sc in range(S // P):
                            pso = pp3.tile([P, SB], f32, tag="o")
                            for dc in range(HPC):
                                nc.tensor.matmul(
                                    pso, lhsT=ctx_sb[:, dc, sc * P:(sc + 1) * P],
                                    rhs=wo_sl[:, dc, :],
                                    start=(dc == 0), stop=(dc == HPC - 1))
                            st = stp.tile([P, SB], f32, tag="st")
                            nc.scalar.activation(out=st, in_=pso, func=AF.Copy)
                            nc.sync.dma_start(
                                out=outp[:][sc, :, eb * SB:(eb + 1) * SB], in_=st)

    nc.compile()
    return nc


def _get_program():
    global _PROGRAM
    if _PROGRAM is None:
        _PROGRAM = _build_program()
    return _PROGRAM


def _host_prep(hidden_states, cos, sin, attention_mask, Wq, Wk, Wv, Wo):
    """Build the 8 per-core input maps."""
    WqT = np.ascontiguousarray(Wq.T).astype(BF16)          # [H, NH*HD]
    WkT = np.ascontiguousarray(Wk.T).astype(BF16)          # [H, NKV*HD]
    WvT = np.ascontiguousarray(Wv.T).astype(BF16)
    WoT = np.ascontiguousarray(Wo.T).astype(BF16)          # [NH*HD, H]

    # rotate-half matrix, stored as lhsT (lhsT[k, m] = R[m, k])
    R = np.zeros((P, P), np.float32)
    half = HD // 2
    for m in range(half):
        R[m, m + half] = -1.0
    for m in range(half, HD):
        R[m, m - half] = 1.0
    rotT = np.ascontiguousarray(R.T).astype(BF16)

    hT_b, cosT_b, sinT_b, expmT_b = [], [], [], []
    for b in range(B):
        hT_b.append(np.ascontiguousarray(hidden_states[b].T).astype(BF16)
                    .reshape(KC, P, S))
        cosT_b.append(np.ascontiguousarray(cos[b].T).astype(BF16))
        sinT_b.append(np.ascontiguousarray(sin[b].T).astype(BF16))
        with np.errstate(over="ignore", under="ignore"):
            em = np.exp(attention_mask[b, 0].astype(np.float64)).astype(np.float32)
        expmT_b.append(np.ascontiguousarray(em.T).astype(BF16).reshape(TC, P, S))

    in_maps = []
    for c in range(NCORES):
        b = c // 4
        h0 = HPC * (c % 4)          # first global q-head
        g0 = GPC * (c % 4)          # first global kv-head
        in_maps.append({
            "hT": hT_b[b],
            "wqT": np.ascontiguousarray(
                WqT[:, h0 * HD:(h0 + HPC) * HD]).reshape(KC, P, HPC * HD),
            "wkT": np.ascontiguousarray(
                WkT[:, g0 * HD:(g0 + GPC) * HD]).reshape(KC, P, GPC * HD),
            "wvT": np.ascontiguousarray(
                WvT[:, g0 * HD:(g0 + GPC) * HD]).reshape(KC, P, GPC * HD),
            "woT": np.ascontiguousarray(
                WoT[h0 * HD:(h0 + HPC) * HD, :]).reshape(HPC, P, H),
            "cosT": cosT_b[b],
            "sinT": sinT_b[b],
            "expmT": expmT_b[b],
            "rotT": rotT,
        })
    return in_maps


def kernel(hidden_states, cos, sin, attention_mask, Wq, Wk, Wv, Wo,
           _trace=False):
    from concourse.bass_utils import run_bass_kernel_spmd

    nc = _get_program()
    in_maps = _host_prep(hidden_states, cos, sin, attention_mask, Wq, Wk, Wv, Wo)
    res = run_bass_kernel_spmd(nc, in_maps, list(range(NCORES)), trace=_trace)
    results = res.results

    attn_output = np.zeros((B, S, H), np.float32)
    aw_t = np.empty((B, NH, S, S), np.float32)   # [b, h, t, s]
    for c in range(NCORES):
        b = c // 4
        h0 = HPC * (c % 4)
        attn_output[b] += np.asarray(results[c]["outp"], np.float32).reshape(S, H)
        aw_t[b, h0:h0 + HPC] = (
            np.asarray(results[c]["attnT"]).reshape(HPC, S, S).astype(np.float32))
    attn_weights = aw_t.transpose(0, 1, 3, 2)    # view: [b, h, s, t]
    if _trace:
        kernel._last_exec_time_ns = res.exec_time_ns
    return attn_output, attn_weights


# revision 4
# speedup vs baseline: 1.3605x; 1.3605x over previous
"""Llama GQA attention (B=2, S=2048, H=4096, 32 q-heads / 8 kv-heads, RoPE,
additive mask, returns (attn_output, attn_weights)) on 8 Trainium2
NeuronCores.

Sharding: core c handles batch b = c//4 and kv-head groups 2*(c%4), 2*(c%4)+1
(= 8 contiguous q-heads).  Tensor-parallel across heads per the hint; the
output projection produces per-core partial sums reduced on the host.

Device-side formulation (per core):
  - All matmuls in bf16 with fp32 PSUM accumulation.
  - hidden^T / weights^T pre-transposed + bf16-cast on the host so every DMA
    is natural-layout.
  - scores are computed TRANSPOSED: S^T[t, s], so the attn@V matmul consumes
    the exp tiles directly (contraction dim t on partitions) and no on-chip
    transpose of the 33M-element attention tensor is needed.
  - The additive mask is applied multiplicatively as exp(mask) (host
    precomputed) — exact for {0, -inf} masks, mathematically identical for
    finite masks, and keeps the ACT exp input small without a max pass.
  - Row sums: DVE bf16 add-tree over the t-chunks, then an all-ones [128,128]
    matmul which both reduces across partitions and replicates the result to
    all 128 partitions; 1/sum via ACT exp(-ln(sum)).
  - attn_weights are written transposed [h, t, s] in bf16; the host casts to
    f32 and returns a transposed view.
  - If the mask is exactly causal (host check), fully-masked t-chunks skip
    matmul/exp entirely (zeros DMA'd from a constant tile) and only the 4
    diagonal chunks get the mask multiply.
  - The output projection runs inside the s-block loop so TensorE stays busy
    while ACT/DVE/GPSIMD chew the softmax work of the next block.
"""

import numpy as np
import ml_dtypes

B, S, H = 2, 2048, 4096
NH, NKV, HD = 32, 8, 128
REP = NH // NKV
SCALE = HD ** -0.5
P = 128
NCORES = 8
KC = H // P            # 32 contraction chunks for projections
TC = S // P            # 16 key/t chunks
SB = 512               # s block for attention phase
NSB = S // SB          # 4
SHALF = S // 2         # 1024, hidden^T streamed in halves
HPC = 8                # q-heads per core
GPC = 2                # kv groups per core

BF16 = ml_dtypes.bfloat16

_PROGRAMS = {}


def _build_program(causal):
    import concourse.bass as bass
    import concourse.tile as tile
    from concourse import bacc, mybir

    f32 = mybir.dt.float32
    bf16 = mybir.dt.bfloat16
    AF = mybir.ActivationFunctionType

    nc = bacc.Bacc("TRN2", target_bir_lowering=False)

    # ---- I/O ----
    hT = nc.declare_dram_parameter("hT", [KC, P, S], bf16, isOutput=False)
    wqT = nc.declare_dram_parameter("wqT", [KC, P, HPC * HD], bf16, isOutput=False)
    wkT = nc.declare_dram_parameter("wkT", [KC, P, GPC * HD], bf16, isOutput=False)
    wvT = nc.declare_dram_parameter("wvT", [KC, P, GPC * HD], bf16, isOutput=False)
    woT = nc.declare_dram_parameter("woT", [HPC, P, H], bf16, isOutput=False)
    cosT = nc.declare_dram_parameter("cosT", [P, S], bf16, isOutput=False)
    sinT = nc.declare_dram_parameter("sinT", [P, S], bf16, isOutput=False)
    expmT = nc.declare_dram_parameter("expmT", [TC, P, S], bf16, isOutput=False)
    rotT = nc.declare_dram_parameter("rotT", [P, P], bf16, isOutput=False)
    attnT = nc.declare_dram_parameter("attnT", [HPC, TC, P, S], bf16, isOutput=True)
    outp = nc.declare_dram_parameter("outp", [S // P, P, H], f32, isOutput=True)

    with tile.TileContext(nc) as tctx:
        with (
            tctx.tile_pool(name="singles", bufs=1) as singles,
            tctx.tile_pool(name="persist", bufs=1) as persist,
        ):
            rot_sb = singles.tile([P, P], bf16)
            nc.sync.dma_start(out=rot_sb, in_=rotT[:])
            ones_sb = singles.tile([P, P], bf16)
            nc.vector.memset(ones_sb, 1.0)

            # persistent activations (bf16)
            q_sb = persist.tile([P, HPC, S], bf16)    # Q^T rope'd  [d, head, s]
            k_sb = persist.tile([P, GPC, S], bf16)    # K^T rope'd  [d, group, t]
            v_sb = persist.tile([P, TC, GPC * HD], bf16)  # V natural [t_in, chunk, d]
            if causal:
                zero_sb = persist.tile([P, 12, SB], bf16)
                nc.vector.memset(zero_sb, 0.0)

            # ================= Phase 1: projections + RoPE =================
            with (
                tctx.tile_pool(name="ph1_h", bufs=KC) as hpool,
                tctx.tile_pool(name="ph1_wq", bufs=2) as wqpool,
                tctx.tile_pool(name="ph1_wres", bufs=1) as wres,
                tctx.tile_pool(name="ph1_cs", bufs=1) as cspool,
                tctx.tile_pool(name="ph1_tmp", bufs=3) as tmp1,
                tctx.tile_pool(name="ph1_ps", bufs=2, space="PSUM") as pp1,
                tctx.tile_pool(name="ph1_psr", bufs=2, space="PSUM") as ppr,
            ):
                cos_sb = cspool.tile([P, S], bf16)
                nc.sync.dma_start(out=cos_sb, in_=cosT[:])
                sin_sb = cspool.tile([P, S], bf16)
                nc.sync.dma_start(out=sin_sb, in_=sinT[:])
                wk_sb = wres.tile([P, KC, GPC * HD], bf16)
                nc.sync.dma_start(out=wk_sb, in_=wkT[:].rearrange("c p d -> p c d"))
                wv_sb = wres.tile([P, KC, GPC * HD], bf16)
                nc.sync.dma_start(out=wv_sb, in_=wvT[:].rearrange("c p d -> p c d"))

                def rope(dst, psrc, s0):
                    # dst (bf16, [P, SB]) holds the raw projection; psrc is a
                    # free psum slot for the rotate matmul.
                    nc.tensor.matmul(psrc, lhsT=rot_sb, rhs=dst, start=True, stop=True)
                    t0 = tmp1.tile([P, SB], bf16, tag="t0")
                    nc.vector.tensor_mul(out=t0, in0=dst, in1=cos_sb[:, s0:s0 + SB])
                    nc.vector.tensor_mul(out=dst, in0=psrc, in1=sin_sb[:, s0:s0 + SB])
                    nc.vector.tensor_add(out=dst, in0=dst, in1=t0)

                for sh in range(2):
                    hcs = []
                    for k in range(KC):
                        hc = hpool.tile([P, SHALF], bf16, tag="hc")
                        nc.sync.dma_start(
                            out=hc, in_=hT[:][k, :, sh * SHALF:(sh + 1) * SHALF])
                        hcs.append(hc)

                    # Q projections (8 head-chunks)
                    for gh in range(HPC):
                        wq_sl = wqpool.tile([P, KC, HD], bf16, tag="wq")
                        nc.sync.dma_start(
                            out=wq_sl,
                            in_=wqT[:][:, :, gh * HD:(gh + 1) * HD].rearrange("c p d -> p c d"))
                        for sb in range(SHALF // SB):
                            s0 = sh * SHALF + sb * SB
                            ps = pp1.tile([P, SB], f32, tag="proj")
                            for k in range(KC):
                                nc.tensor.matmul(
                                    ps, lhsT=wq_sl[:, k, :],
                                    rhs=hcs[k][:, sb * SB:(sb + 1) * SB],
                                    start=(k == 0), stop=(k == KC - 1))
                            dst = q_sb[:, gh, s0:s0 + SB]
                            nc.scalar.activation(out=dst, in_=ps, func=AF.Copy)
                            pr = ppr.tile([P, SB], f32, tag="rot")
                            rope(dst, pr, s0)

                    # K projections (2 groups)
                    for g in range(GPC):
                        for sb in range(SHALF // SB):
                            s0 = sh * SHALF + sb * SB
                            ps = pp1.tile([P, SB], f32, tag="proj")
                            for k in range(KC):
                                nc.tensor.matmul(
                                    ps, lhsT=wk_sb[:, k, g * HD:(g + 1) * HD],
                                    rhs=hcs[k][:, sb * SB:(sb + 1) * SB],
                                    start=(k == 0), stop=(k == KC - 1))
                            dst = k_sb[:, g, s0:s0 + SB]
                            nc.scalar.activation(out=dst, in_=ps, func=AF.Copy)
                            pr = ppr.tile([P, SB], f32, tag="rot")
                            rope(dst, pr, s0)

                    # V projection, natural layout [t, d], both groups at once
                    for tb in range(SHALF // P):
                        tg = sh * (SHALF // P) + tb
                        psv = pp1.tile([P, GPC * HD], f32, tag="projv")
                        for k in range(KC):
                            nc.tensor.matmul(
                                psv, lhsT=hcs[k][:, tb * P:(tb + 1) * P],
                                rhs=wv_sb[:, k, :],
                                start=(k == 0), stop=(k == KC - 1))
                        nc.scalar.activation(out=v_sb[:, tg, :], in_=psv, func=AF.Copy)

            # persistent ctx (allocated after phase-1 pools close)
            with tctx.tile_pool(name="ctxp", bufs=1) as ctxp:
                ctx_sb = ctxp.tile([P, HPC, S], bf16)   # ctx^T [d, head, s]

                # ============ Phase 2: attention + output projection ============
                NEXPM = 4 if causal else TC
                with (
                    tctx.tile_pool(name="ph2_expm", bufs=2) as expmp,
                    tctx.tile_pool(name="ph2_exp", bufs=2) as expp,
                    tctx.tile_pool(name="ph2_tree", bufs=1) as tree,
                    tctx.tile_pool(name="ph2_wo", bufs=2) as wop,
                    tctx.tile_pool(name="ph2_st", bufs=3) as stp,
                    tctx.tile_pool(name="ph2_ps", bufs=3, space="PSUM") as pps,
                    tctx.tile_pool(name="ph2_psc", bufs=2, space="PSUM") as ppc,
                    tctx.tile_pool(name="ph2_pssum", bufs=1, space="PSUM") as ppsum,
                    tctx.tile_pool(name="ph2_pso", bufs=2, space="PSUM") as ppo,
                ):
                    for sb4 in range(NSB):
                        s0 = sb4 * SB
                        nact = (s0 // P) + 4 if causal else TC  # active t-chunks
                        c0 = (s0 // P) if causal else 0         # first partial chunk
                        expm_sb = expmp.tile([P, NEXPM, SB], bf16, tag="expm")
                        nc.sync.dma_start(
                            out=expm_sb,
                            in_=expmT[:][c0:c0 + NEXPM, :, s0:s0 + SB]
                                .rearrange("c p s -> p c s"))
                        for gh in range(HPC):
                            g = gh // REP
                            exp_sb = expp.tile([P, TC, SB], bf16, tag="exp")
                            for t in range(nact):
                                pss = pps.tile([P, SB], f32, tag="scores")
                                nc.tensor.matmul(
                                    pss, lhsT=k_sb[:, g, t * P:(t + 1) * P],
                                    rhs=q_sb[:, gh, s0:s0 + SB],
                                    start=True, stop=True)
                                nc.scalar.activation(
                                    out=exp_sb[:, t, :], in_=pss, func=AF.Exp,
                                    scale=SCALE)
                            # mask (multiplicative exp(mask)) on gpsimd
                            nc.gpsimd.tensor_mul(
                                out=exp_sb[:, c0:c0 + NEXPM, :],
                                in0=exp_sb[:, c0:c0 + NEXPM, :], in1=expm_sb)
                            # bf16 add-tree over the active chunks
                            cur, n = exp_sb, nact
                            while n > 1:
                                h = n // 2
                                t2 = tree.tile([P, h, SB], bf16, tag=f"tl{h}")
                                nc.vector.tensor_add(
                                    t2, cur[:, 0:h, :], cur[:, h:2 * h, :])
                                if n % 2:
                                    nc.vector.tensor_add(
                                        t2[:, 0, :], t2[:, 0, :], cur[:, 2 * h, :])
                                cur, n = t2, h
                            acc = cur[:, 0, :]
                            # cross-partition sum replicated to 128 partitions
                            pssum = ppsum.tile([P, SB], f32, tag="sum")
                            nc.tensor.matmul(pssum, lhsT=ones_sb, rhs=acc,
                                             start=True, stop=True)
                            lnt = tree.tile([P, SB], f32, tag="lnt")
                            nc.scalar.activation(out=lnt, in_=pssum, func=AF.Ln)
                            rb = tree.tile([P, SB], bf16, tag="rb")
                            nc.scalar.activation(out=rb, in_=lnt, func=AF.Exp,
                                                 scale=-1.0)
                            # normalize in place (rb broadcast over chunk dim)
                            nc.vector.tensor_mul(
                                out=exp_sb[:, 0:nact, :], in0=exp_sb[:, 0:nact, :],
                                in1=rb[:, None, :].to_broadcast((P, nact, SB)))
                            # attn @ V  -> ctx^T [d, s]
                            psc = ppc.tile([P, SB], f32, tag="ctx")
                            for t in range(nact):
                                nc.tensor.matmul(
                                    psc, lhsT=v_sb[:, t, g * HD:(g + 1) * HD],
                                    rhs=exp_sb[:, t, :],
                                    start=(t == 0), stop=(t == nact - 1))
                            nc.scalar.activation(out=ctx_sb[:, gh, s0:s0 + SB],
                                                 in_=psc, func=AF.Copy)
                            # write normalized attn (transposed layout)
                            nc.sync.dma_start(
                                out=attnT[:][gh, 0:nact, :, s0:s0 + SB]
                                    .rearrange("c p s -> p c s"),
                                in_=exp_sb[:, 0:nact, :])
                            if causal and nact < TC:
                                nc.sync.dma_start(
                                    out=attnT[:][gh, nact:TC, :, s0:s0 + SB]
                                        .rearrange("c p s -> p c s"),
                                    in_=zero_sb[:, 0:TC - nact, :])

                        # ---- output projection for this s block ----
                        for eb in range(H // SB):
                            wo_sl = wop.tile([P, HPC, SB], bf16, tag="wo")
                            nc.sync.dma_start(
                                out=wo_sl,
                                in_=woT[:][:, :, eb * SB:(eb + 1) * SB]
                                    .rearrange("c p e -> p c e"))
                            for sc in range(SB // P):
                                scg = sb4 * (SB // P) + sc
                                pso = ppo.tile([P, SB], f32, tag="o")
                                for dc in range(HPC):
                                    nc.tensor.matmul(
                                        pso,
                                        lhsT=ctx_sb[:, dc, scg * P:(scg + 1) * P],
                                        rhs=wo_sl[:, dc, :],
                                        start=(dc == 0), stop=(dc == HPC - 1))
                                st = stp.tile([P, SB], f32, tag="st")
                                nc.scalar.activation(out=st, in_=pso, func=AF.Copy)
                                nc.sync.dma_start(
                                    out=outp[:][scg, :, eb * SB:(eb + 1) * SB],
                                    in_=st)

    nc.compile()
    return nc


def _get_program(causal):
    if causal not in _PROGRAMS:
        _PROGRAMS[causal] = _build_program(causal)
    return _PROGRAMS[causal]


def _is_causal(attention_mask):
    big_neg = np.finfo(np.float32).min
    causal = np.where(np.tril(np.ones((S, S), bool)), np.float32(0.0),
                      big_neg).astype(np.float32)
    for b in range(attention_mask.shape[0]):
        if not np.array_equal(np.asarray(attention_mask[b, 0], np.float32), causal):
            return False
    return True


def _host_prep(hidden_states, cos, sin, attention_mask, Wq, Wk, Wv, Wo):
    """Build the 8 per-core input maps."""
    WqT = np.ascontiguousarray(Wq.T).astype(BF16)          # [H, NH*HD]
    WkT = np.ascontiguousarray(Wk.T).astype(BF16)          # [H, NKV*HD]
    WvT = np.ascontiguousarray(Wv.T).astype(BF16)
    WoT = np.ascontiguousarray(Wo.T).astype(BF16)          # [NH*HD, H]

    # rotate-half matrix, stored as lhsT (lhsT[k, m] = R[m, k])
    R = np.zeros((P, P), np.float32)
    half = HD // 2
    for m in range(half):
        R[m, m + half] = -1.0
    for m in range(half, HD):
        R[m, m - half] = 1.0
    rotT = np.ascontiguousarray(R.T).astype(BF16)

    hT_b, cosT_b, sinT_b, expmT_b = [], [], [], []
    for b in range(B):
        hT_b.append(np.ascontiguousarray(hidden_states[b].T).astype(BF16)
                    .reshape(KC, P, S))
        cosT_b.append(np.ascontiguousarray(cos[b].T).astype(BF16))
        sinT_b.append(np.ascontiguousarray(sin[b].T).astype(BF16))
        with np.errstate(over="ignore", under="ignore"):
            em = np.exp(attention_mask[b, 0].astype(np.float64)).astype(np.float32)
        expmT_b.append(np.ascontiguousarray(em.T).astype(BF16).reshape(TC, P, S))

    in_maps = []
    for c in range(NCORES):
        b = c // 4
        h0 = HPC * (c % 4)          # first global q-head
        g0 = GPC * (c % 4)          # first global kv-head
        in_maps.append({
            "hT": hT_b[b],
            "wqT": np.ascontiguousarray(
                WqT[:, h0 * HD:(h0 + HPC) * HD]).reshape(KC, P, HPC * HD),
            "wkT": np.ascontiguousarray(
                WkT[:, g0 * HD:(g0 + GPC) * HD]).reshape(KC, P, GPC * HD),
            "wvT": np.ascontiguousarray(
                WvT[:, g0 * HD:(g0 + GPC) * HD]).reshape(KC, P, GPC * HD),
            "woT": np.ascontiguousarray(
                WoT[h0 * HD:(h0 + HPC) * HD, :]).reshape(HPC, P, H),
            "cosT": cosT_b[b],
            "sinT": sinT_b[b],
            "expmT": expmT_b[b],
            "rotT": rotT,
        })
    return in_maps


def kernel(hidden_states, cos, sin, attention_mask, Wq, Wk, Wv, Wo,
           _trace=False):
    from concourse.bass_utils import run_bass_kernel_spmd

    causal = _is_causal(np.asarray(attention_mask))
    nc = _get_program(causal)
    in_maps = _host_prep(hidden_states, cos, sin, attention_mask, Wq, Wk, Wv, Wo)
    res = run_bass_kernel_spmd(nc, in_maps, list(range(NCORES)), trace=_trace)
    results = res.results

    attn_output = np.zeros((B, S, H), np.float32)
    aw_t = np.empty((B, NH, S, S), np.float32)   # [b, h, t, s]
    for c in range(NCORES):
        b = c // 4
        h0 = HPC * (c % 4)
        attn_output[b] += np.asarray(results[c]["outp"], np.float32).reshape(S, H)
        aw_t[b, h0:h0 + HPC] = (
            np.asarray(results[c]["attnT"]).reshape(HPC, S, S).astype(np.float32))
    attn_weights = aw_t.transpose(0, 1, 3, 2)    # view: [b, h, s, t]
    if _trace:
        kernel._last_exec_time_ns = res.exec_time_ns
    return attn_output, attn_weights
